# revision 24
# baseline (speedup 1.0000x reference)
"""Trainium2 Bass kernel for 16-head causal self-attention (KaplanAttention).

Problem: x [2, 2048, 1024], torch-style weights W_q/W_k/W_v/W_o [1024, 1024].
  q/k/v = (x @ W.T) split into 16 heads of 64; causal softmax(q k^T / 8) @ v;
  concat heads; out = attn_out @ W_o.T.

Sharding (8 cores): core c handles batch b = c // 4 and head group g = c % 4
(heads 4g..4g+3). Each core computes its 4 heads' attention output and a
partial output projection against the matching 256 columns of W_o; the host
sums the 4 fp16 partials per batch (the "all-reduce" of the row-sharded W_o).

Layouts (matmul operands fp16, accumulation fp32):
  xT  [1024, 2048] = x[b].T                      e on partitions
  wqT/wkT/wvT [1024, 256] = W[256g:256g+256].T   e on partitions
  woT [256, 1024] = W_o[:, 256g:256g+256].T      d on partitions
  QT/KT [128, 2, 2048]: head pair hp, head hi at partitions 64*hi
  V     [128, 16, 4, 65]: j-chunk k, head h -> [V_h | ones] (ones col gives
        the softmax denominator for free from the same AV matmul)

Schedule: column blocks (hp, t) of 512 queries processed in the order
(0,0),(0,1),(1,1),(0,2),(1,2),(0,3),(1,3),(1,0) so the kernel ends on the
smallest exp burst. Within a block, per j-tile: row-tiled score matmul pair
(K=64 at partitions 0/64, concurrent), exp on ACT (scale 1/8 fused), causal
mask mul on the diagonal 128-block, then the AV matmul pair for the PREVIOUS
j-tile accumulates [V_h | ones]^T @ U^T into a per-(block,hi) psum [65, 512]
(row 64 = Z).  QKV projections and the final projection run as generator
"fillers" pumped between score chunks so TensorE always has queued work.

Normalization per block: Z rows hop to partition 0 via a DVE copy (output
partition start may differ from the shared input start), reciprocal +
fp16 cast on DVE, then a K=1 PE matmul against a ones row broadcasts 1/Z
to 64 partitions in PSUM; one DVE copy + one DVE mul produce the normalized
out^T directly from the AV psum (no gpsimd, no DMA hop, no outU staging).
"""

from collections import deque

import numpy as np

from concourse import bass_utils, mybir, tile
from concourse import bacc

S = 2048
D = 1024
HPC = 4        # heads per core
DK = 64
DC = HPC * DK  # 256 d-columns per core
NCORES = 8
EC = D // 128  # 8 e-chunks
NJT = S // 128  # 16 j-tiles
NT = S // 512   # 4 query column blocks

FP16 = mybir.dt.float16
FP32 = mybir.dt.float32

BLOCKS = [(0, 0), (0, 1), (1, 1), (0, 2), (1, 2), (0, 3), (1, 3), (1, 0)]


def _build():
    nc = bacc.Bacc("TRN2", target_bir_lowering=False, debug=False)

    xT_d = nc.dram_tensor("xT", [D, S], FP16, kind="ExternalInput")
    wq_d = nc.dram_tensor("wqT", [D, DC], FP16, kind="ExternalInput")
    wk_d = nc.dram_tensor("wkT", [D, DC], FP16, kind="ExternalInput")
    wv_d = nc.dram_tensor("wvT", [D, DC], FP16, kind="ExternalInput")
    wo_d = nc.dram_tensor("woT", [DC, D], FP16, kind="ExternalInput")
    mask_d = nc.dram_tensor("mask", [128, 2, 128], FP16, kind="ExternalInput")
    out_d = nc.dram_tensor("out", [S, D], FP16, kind="ExternalOutput")

    with tile.TileContext(nc) as tc:
        with (
            tc.tile_pool(name="const", bufs=1) as const,
            tc.tile_pool(name="work", bufs=1) as work,
            tc.tile_pool(name="ut", bufs=4) as utp,
            tc.tile_pool(name="outs", bufs=3) as outs,
            tc.tile_pool(name="norm", bufs=2) as normp,
            tc.tile_pool(name="psS", bufs=2, space="PSUM") as psS,
            tc.tile_pool(name="psA", bufs=2, space="PSUM") as psA,
            tc.tile_pool(name="psV", bufs=2, space="PSUM") as psV,
        ):
            # ---- load inputs (big DMAs; order = first-needed first) ----
            scr = const.tile([1, 16], FP32)
            scr2 = const.tile([1, 16], FP32)
            mask = const.tile([128, 2, 128], FP16)
            wq = const.tile([128, EC, DC], FP16)
            wk = const.tile([128, EC, DC], FP16)
            wv = const.tile([128, EC, DC], FP16)
            xT = const.tile([128, EC, S], FP16)
            wo = const.tile([128, 2, D], FP16)
            # head loads fan out across the two HWDGE queues (gpsimd/SWDGE
            # pays ~2us extra completion latency - avoid for critical loads)
            xT_r = xT_d.rearrange("(c p) s -> p c s", p=128)
            nc.scalar.dma_start(
                out=wq, in_=wq_d.rearrange("(c p) d -> p c d", p=128)
            )
            nc.sync.dma_start(out=xT[:, 0:4, 0:512], in_=xT_r[:, 0:4, 0:512])
            nc.scalar.dma_start(out=xT[:, 4:8, 0:512], in_=xT_r[:, 4:8, 0:512])
            nc.sync.dma_start(
                out=wk, in_=wk_d.rearrange("(c p) d -> p c d", p=128)
            )
            # warm the exp table while the loads run (after DMAs on scalar)
            nc.vector.memset(scr, 0.0)
            nc.scalar.activation(
                out=scr2, in_=scr, func=mybir.ActivationFunctionType.Exp
            )
            nc.sync.dma_start(out=wv, in_=wv_d.rearrange("(c p) d -> p c d", p=128))
            nc.sync.dma_start(out=mask, in_=mask_d[:, :, :])
            nc.sync.dma_start(out=xT[:, :, 512:1024], in_=xT_r[:, :, 512:1024])
            nc.sync.dma_start(out=xT[:, :, 1024:1536], in_=xT_r[:, :, 1024:1536])
            nc.sync.dma_start(out=xT[:, :, 1536:2048], in_=xT_r[:, :, 1536:2048])
            nc.sync.dma_start(out=wo, in_=wo_d.rearrange("(c p) d -> p c d", p=128))

            QT = work.tile([128, 2, S], FP16)
            KT = work.tile([128, 2, S], FP16)
            V = work.tile([128, NJT, HPC, 65], FP16)
            outN = work.tile([128, 2, S], FP16)    # normalized out^T
            ones16 = work.tile([1, 64], FP16)      # bcast lhsT (partition 0)
            dumw = work.tile([128, 512], FP16)     # HAM warm-up operand

            nc.vector.memset(V[:, :, :, 64:65], 1.0)
            nc.vector.memset(ones16, 1.0)
            nc.vector.memset(dumw, 0.0)

            # ---- HAM warm-up: keep the PE busy while input DMAs fly so the
            # clock gate is already at 2.4 GHz when real matmuls start ----
            for r in range(30):
                psw = psS.tile([128, 2, 512], FP32, tag="score", name="psw")
                nc.tensor.matmul(
                    psw[:, 0, :], dumw[:, 0:128], dumw, start=True, stop=True
                )

            # ---- generator-based TensorE fillers -------------------------
            # filler: dependency-urgent work (QKV chains, norm tails).
            # backlog: flexible work (final projections) drained when the
            # urgent queue is empty, i.e. during ACT-bound stretches.
            filler = deque()
            backlog = deque()

            def pump(n=1):
                for _ in range(n):
                    while filler:
                        try:
                            next(filler[0])
                            break
                        except StopIteration:
                            filler.popleft()
                    else:
                        while backlog:
                            try:
                                next(backlog[0])
                                break
                            except StopIteration:
                                backlog.popleft()
                        else:
                            break

            def drain_fillers():
                while filler or backlog:
                    q = filler if filler else backlog
                    try:
                        next(q[0])
                    except StopIteration:
                        q.popleft()

            def run_now(gen):
                for _ in gen:
                    pass

            def qk_chain(w_t, dst, hp, st):
                ps = psV.tile([128, 512], FP32, tag="proj")
                for c in range(EC):
                    nc.tensor.matmul(
                        ps,
                        w_t[:, c, 128 * hp : 128 * (hp + 1)],
                        xT[:, c, 512 * st : 512 * (st + 1)],
                        start=(c == 0),
                        stop=(c == EC - 1),
                    )
                    if c % 2 == 1 and c < EC - 1:
                        yield
                nc.vector.tensor_copy(
                    out=dst[:, hp, 512 * st : 512 * (st + 1)], in_=ps
                )

            def v_chain(jt):
                ps = psV.tile([128, 512], FP32, tag="proj")
                psd = ps[:, 0:DC]
                for c in range(EC):
                    nc.tensor.matmul(
                        psd,
                        xT[:, c, 128 * jt : 128 * (jt + 1)],
                        wv[:, c, :],
                        start=(c == 0),
                        stop=(c == EC - 1),
                    )
                    if c == 3:
                        yield
                nc.vector.tensor_copy(
                    out=V[:, jt, :, 0:64],
                    in_=psd.rearrange("p (h d) -> p h d", h=HPC),
                )

            def final_proj(st, use_scalar=False, tail=False):
                ob = outs.tile([128, D], FP16, tag="ob")
                for mt in range(2):
                    psf = psV.tile([128, 512], FP32, tag="proj")
                    for hp in range(2):
                        nc.tensor.matmul(
                            psf,
                            outN[:, hp, 128 * st : 128 * (st + 1)],
                            wo[:, hp, 512 * mt : 512 * (mt + 1)],
                            start=(hp == 0),
                            stop=(hp == 1),
                        )
                    if tail and mt == 1:
                        # alternate engines at the tail so the two halves'
                        # psum->sbuf copies run in parallel
                        nc.vector.tensor_copy(
                            out=ob[:, 512 * mt : 512 * (mt + 1)], in_=psf
                        )
                    elif use_scalar or tail:
                        # ScalarE is idle around the small t=0 blocks
                        nc.scalar.copy(
                            out=ob[:, 512 * mt : 512 * (mt + 1)], in_=psf
                        )
                    else:
                        nc.vector.tensor_copy(
                            out=ob[:, 512 * mt : 512 * (mt + 1)], in_=psf
                        )
                    if tail:
                        nc.sync.dma_start(
                            out=out_d[128 * st : 128 * (st + 1),
                                      512 * mt : 512 * (mt + 1)],
                            in_=ob[:, 512 * mt : 512 * (mt + 1)],
                        )
                    yield
                if not tail:
                    nc.sync.dma_start(
                        out=out_d[128 * st : 128 * (st + 1), :], in_=ob
                    )

            def norm_tail(hp, t, psa, Zrh):
                # PE bcast of 1/Z via K=1 matmul against a ones row, then
                # one copy + one mul per head produce normalized out^T
                # straight from the AV psum.  Run inline at jt==1 of the
                # following block so the DVE reciprocal chain has a head
                # start before the bcast matmul enters the in-order PE queue.
                zbs = normp.tile([64, 2, 512], FP16, tag="zbs")
                for hi in range(2):
                    zb = psV.tile([128, 512], FP32, tag="proj")
                    nc.tensor.matmul(
                        zb[0:64, :],
                        ones16[0:1, 0:64],
                        Zrh[0:1, hi, :],
                        start=True,
                        stop=True,
                    )
                    nc.vector.tensor_copy(out=zbs[:, hi, :], in_=zb[0:64, :])
                    nc.vector.tensor_mul(
                        outN[64 * hi : 64 * hi + 64, hp,
                             512 * t : 512 * (t + 1)],
                        psa[hi][0:64, :],
                        zbs[:, hi, :],
                    )
                    yield

            def emit_av(psa, hp, jt, t, njt, UT):
                off = max(0, 128 * jt - 512 * t)
                cn = 512 - off
                for hi in range(2):
                    h = 2 * hp + hi
                    nc.tensor.matmul(
                        psa[hi][:, off : off + cn],
                        V[:, jt, h, :],
                        UT[:, hi, off : off + cn],
                        start=(jt == 0),
                        stop=(jt == njt - 1),
                    )

            def colblock(hp, t, prev_norm=None):
                njt = 4 * t + 4
                psa = [
                    psA.tile([65, 512], FP32, tag="av", name=f"psa{hi}")
                    for hi in range(2)
                ]
                pending = deque()
                for jt in range(njt):
                    # fillers + the ready AV go in the queue BEFORE the
                    # (exp-gated) score matmul so the in-order PE has work
                    # while it waits for ACT.  Exception: the first two jts
                    # go straight to scores so ACT never starves across the
                    # block boundary.
                    if jt >= 2:
                        pump(4 if t == 0 else 3)
                        if len(pending) >= 2:
                            pjt, pUT = pending.popleft()
                            emit_av(psa, hp, pjt, t, njt, pUT)
                    off = max(0, 128 * jt - 512 * t)
                    cn = 512 - off
                    ps = psS.tile([128, 2, 512], FP32, tag="score")
                    for hi in range(2):
                        ho = 64 * hi
                        nc.tensor.matmul(
                            ps[:, hi, 0:cn],
                            KT[ho : ho + 64, hp, 128 * jt : 128 * (jt + 1)],
                            QT[ho : ho + 64, hp, 512 * t + off : 512 * (t + 1)],
                            start=True,
                            stop=True,
                        )
                    UT = utp.tile([128, 2, 512], FP16, tag="ut")
                    nc.scalar.activation(
                        out=UT[:, :, off : off + cn],
                        in_=ps[:, :, 0:cn],
                        func=mybir.ActivationFunctionType.Exp,
                        scale=0.125,
                    )
                    if jt >= 4 * t:
                        # diagonal 128-block: causal {0,1} mask, both heads
                        # (gpsimd, to keep DVE off the exp->AV critical path)
                        nc.gpsimd.tensor_mul(
                            UT[:, :, off : off + 128],
                            UT[:, :, off : off + 128],
                            mask,
                        )
                    if jt == 1 and prev_norm is not None:
                        # emit the previous block's norm muls, THEN unlock
                        # its final projections (emission order = queue
                        # order; fp must never precede its outN writer)
                        pgen, phq, pt = prev_norm
                        run_now(pgen)
                        queue_fp(phq, pt)
                    pending.append((jt, UT))
                while pending:
                    pjt, pUT = pending.popleft()
                    emit_av(psa, hp, pjt, t, njt, pUT)
                # ---- normalization: Z -> partition 0, reciprocal, fp16 ----
                Zs = normp.tile([1, 2, 512], FP32, tag="zs")
                for hi in range(2):
                    nc.vector.tensor_copy(
                        out=Zs[0:1, hi, :], in_=psa[hi][64:65, :]
                    )
                Zr = normp.tile([1, 2, 512], FP32, tag="zr")
                nc.vector.reciprocal_approx_fast(out=Zr, in_=Zs)
                Zrh = normp.tile([1, 2, 512], FP16, tag="zrh")
                nc.vector.tensor_copy(out=Zrh, in_=Zr)
                return norm_tail(hp, t, psa, Zrh)

            # ---- schedule ------------------------------------------------
            done = set()
            tail_fp = []

            def queue_fp(hp, t):
                done.add((hp, t))
                if (1 - hp, t) in done:
                    for st in range(4 * t, 4 * t + 4):
                        if (hp, t) == BLOCKS[-1]:
                            tail_fp.append(st)
                        else:
                            backlog.append(final_proj(st, use_scalar=(t == 3)))

            # head: everything block (0,0) needs, inline
            run_now(qk_chain(wq, QT, 0, 0))
            run_now(qk_chain(wk, KT, 0, 0))
            run_now(v_chain(0))
            run_now(v_chain(1))
            filler.append(v_chain(2))
            filler.append(v_chain(3))

            # fillers each block needs, queued one block ahead
            prefill = {
                (0, 0): [],
                (0, 1): [("qk", 0, 1), ("v", 4), ("v", 5), ("v", 6), ("v", 7)],
                (1, 1): [("qk", 1, 0), ("qk", 1, 1)],
                (0, 2): [("qk", 0, 2), ("v", 8), ("v", 9), ("v", 10), ("v", 11)],
                (1, 2): [("qk", 1, 2)],
                (0, 3): [("qk", 0, 3), ("v", 12), ("v", 13), ("v", 14), ("v", 15)],
                (1, 3): [("qk", 1, 3)],
                (1, 0): [],
            }

            prev_norm = None
            for i, (hp, t) in enumerate(BLOCKS):
                if i + 1 < len(BLOCKS):
                    for item in prefill[BLOCKS[i + 1]]:
                        if item[0] == "qk":
                            _, fhp, fst = item
                            filler.append(qk_chain(wq, QT, fhp, fst))
                            filler.append(qk_chain(wk, KT, fhp, fst))
                        else:
                            filler.append(v_chain(item[1]))
                gen = colblock(hp, t, prev_norm)
                prev_norm = (gen, hp, t)
            gen, hp, t = prev_norm
            run_now(gen)
            queue_fp(hp, t)
            drain_fillers()
            for st in tail_fp:
                run_now(final_proj(st, tail=True))

    nc.compile()
    return nc


_NC = None


def _prep_in_maps(x, W_q, W_k, W_v, W_o):
    x = np.asarray(x, dtype=np.float32)
    W_q = np.asarray(W_q, dtype=np.float32)
    W_k = np.asarray(W_k, dtype=np.float32)
    W_v = np.asarray(W_v, dtype=np.float32)
    W_o = np.asarray(W_o, dtype=np.float32)
    mask01 = np.triu(np.ones((128, 128), dtype=np.float16))
    mask2 = np.ascontiguousarray(np.stack([mask01, mask01], axis=1))
    in_maps = []
    for c in range(NCORES):
        b, g = divmod(c, 4)
        cols = slice(DC * g, DC * (g + 1))
        in_maps.append(
            {
                "xT": np.ascontiguousarray(x[b].T).astype(np.float16),
                "wqT": np.ascontiguousarray(W_q[cols, :].T).astype(np.float16),
                "wkT": np.ascontiguousarray(W_k[cols, :].T).astype(np.float16),
                "wvT": np.ascontiguousarray(W_v[cols, :].T).astype(np.float16),
                "woT": np.ascontiguousarray(W_o[:, cols].T).astype(np.float16),
                "mask": mask2,
            }
        )
    return in_maps


def _run(x, W_q, W_k, W_v, W_o, **spmd_kwargs):
    global _NC
    if _NC is None:
        _NC = _build()
    in_maps = _prep_in_maps(x, W_q, W_k, W_v, W_o)
    res = bass_utils.run_bass_kernel_spmd(
        _NC, in_maps, core_ids=list(range(NCORES)), **spmd_kwargs
    )
    parts = [res.results[c]["out"].astype(np.float32) for c in range(NCORES)]
    out = np.empty((2, S, D), dtype=np.float32)
    for b in range(2):
        out[b] = parts[4 * b] + parts[4 * b + 1] + parts[4 * b + 2] + parts[4 * b + 3]
    return out, res


def kernel(x, W_q, W_k, W_v, W_o):
    out, _ = _run(x, W_q, W_k, W_v, W_o)
    return out


# revision 25
# speedup vs baseline: 1.0401x; 1.0401x over previous
"""Trainium2 Bass kernel for 16-head causal self-attention (KaplanAttention).

Problem: x [2, 2048, 1024], torch-style weights W_q/W_k/W_v/W_o [1024, 1024].
  q/k/v = (x @ W.T) split into 16 heads of 64; causal softmax(q k^T / 8) @ v;
  concat heads; out = attn_out @ W_o.T.

Sharding (8 cores): core c handles batch b = c // 4 and head group g = c % 4
(heads 4g..4g+3). Each core computes its 4 heads' attention output and a
partial output projection against the matching 256 columns of W_o; the host
sums the 4 fp16 partials per batch (the "all-reduce" of the row-sharded W_o).

Layouts (matmul operands fp16, accumulation fp32):
  xT  [1024, 2048] = x[b].T                      e on partitions
  wqT/wkT/wvT [1024, 256] = W[256g:256g+256].T   e on partitions
  woT [256, 1024] = W_o[:, 256g:256g+256].T      d on partitions
  QT/KT [128, 2, 2048]: head pair hp, head hi at partitions 64*hi
  V     [128, 16, 4, 65]: j-chunk k, head h -> [V_h | ones] (ones col gives
        the softmax denominator for free from the same AV matmul)

Schedule: column blocks (hp, t) of 512 queries processed in the order
(0,0),(0,1),(1,1),(0,2),(1,2),(0,3),(1,3),(1,0) so the kernel ends on the
smallest exp burst. Within a block, per j-tile: row-tiled score matmul pair
(K=64 at partitions 0/64, concurrent), exp on ACT (scale 1/8 fused), causal
mask mul on the diagonal 128-block, then the AV matmul pair for the PREVIOUS
j-tile accumulates [V_h | ones]^T @ U^T into a per-(block,hi) psum [65, 512]
(row 64 = Z).  QKV projections and the final projection run as generator
"fillers" pumped between score chunks so TensorE always has queued work.

Normalization per block: Z rows hop to partition 0 via a DVE copy (output
partition start may differ from the shared input start), reciprocal +
fp16 cast on DVE, then a K=1 PE matmul against a ones row broadcasts 1/Z
to 64 partitions in PSUM; one DVE copy + one DVE mul produce the normalized
out^T directly from the AV psum (no gpsimd, no DMA hop, no outU staging).
"""

from collections import deque

import numpy as np

from concourse import bass_utils, mybir, tile
from concourse import bacc

S = 2048
D = 1024
HPC = 4        # heads per core
DK = 64
DC = HPC * DK  # 256 d-columns per core
NCORES = 8
EC = D // 128  # 8 e-chunks
NJT = S // 128  # 16 j-tiles
NT = S // 512   # 4 query column blocks

FP16 = mybir.dt.float16
FP32 = mybir.dt.float32

BLOCKS = [(0, 0), (0, 1), (1, 1), (0, 2), (1, 2), (0, 3), (1, 3), (1, 0)]


def _build():
    nc = bacc.Bacc("TRN2", target_bir_lowering=False, debug=False)

    xT_d = nc.dram_tensor("xT", [D, S], FP16, kind="ExternalInput")
    wq_d = nc.dram_tensor("wqT", [D, DC], FP16, kind="ExternalInput")
    wk_d = nc.dram_tensor("wkT", [D, DC], FP16, kind="ExternalInput")
    wv_d = nc.dram_tensor("wvT", [D, DC], FP16, kind="ExternalInput")
    wo_d = nc.dram_tensor("woT", [DC, D], FP16, kind="ExternalInput")
    mask_d = nc.dram_tensor("mask", [128, 2, 128], FP16, kind="ExternalInput")
    out_d = nc.dram_tensor("out", [S, D], FP16, kind="ExternalOutput")

    with tile.TileContext(nc) as tc:
        with (
            tc.tile_pool(name="const", bufs=1) as const,
            tc.tile_pool(name="work", bufs=1) as work,
            tc.tile_pool(name="ut", bufs=4) as utp,
            tc.tile_pool(name="outs", bufs=3) as outs,
            tc.tile_pool(name="norm", bufs=2) as normp,
            tc.tile_pool(name="psS", bufs=2, space="PSUM") as psS,
            tc.tile_pool(name="psA", bufs=2, space="PSUM") as psA,
            tc.tile_pool(name="psV", bufs=2, space="PSUM") as psV,
        ):
            # ---- load inputs (big DMAs; order = first-needed first) ----
            scr = const.tile([1, 16], FP32)
            scr2 = const.tile([1, 16], FP32)
            mask = const.tile([128, 2, 128], FP16)
            wq = const.tile([128, EC, DC], FP16)
            wk = const.tile([128, EC, DC], FP16)
            wv = const.tile([128, EC, DC], FP16)
            xT = const.tile([128, EC, S], FP16)
            wo = const.tile([128, 2, D], FP16)
            # head loads fan out across the two HWDGE queues (gpsimd/SWDGE
            # pays ~2us extra completion latency - avoid for critical loads)
            xT_r = xT_d.rearrange("(c p) s -> p c s", p=128)
            nc.scalar.dma_start(
                out=wq, in_=wq_d.rearrange("(c p) d -> p c d", p=128)
            )
            nc.sync.dma_start(out=xT[:, 0:4, 0:512], in_=xT_r[:, 0:4, 0:512])
            nc.scalar.dma_start(out=xT[:, 4:8, 0:512], in_=xT_r[:, 4:8, 0:512])
            nc.sync.dma_start(
                out=wk, in_=wk_d.rearrange("(c p) d -> p c d", p=128)
            )
            # warm the exp table while the loads run (after DMAs on scalar)
            nc.vector.memset(scr, 0.0)
            nc.scalar.activation(
                out=scr2, in_=scr, func=mybir.ActivationFunctionType.Exp
            )
            nc.sync.dma_start(out=wv, in_=wv_d.rearrange("(c p) d -> p c d", p=128))
            nc.sync.dma_start(out=mask, in_=mask_d[:, :, :])
            nc.sync.dma_start(out=xT[:, :, 512:1024], in_=xT_r[:, :, 512:1024])
            nc.sync.dma_start(out=xT[:, :, 1024:1536], in_=xT_r[:, :, 1024:1536])
            nc.sync.dma_start(out=xT[:, :, 1536:2048], in_=xT_r[:, :, 1536:2048])
            nc.sync.dma_start(out=wo, in_=wo_d.rearrange("(c p) d -> p c d", p=128))

            QT = work.tile([128, 2, S], FP16)
            KT = work.tile([128, 2, S], FP16)
            V = work.tile([128, NJT, HPC, 65], FP16)
            outN = work.tile([128, 2, S], FP16)    # normalized out^T
            ones16 = work.tile([1, 64], FP16)      # bcast lhsT (partition 0)
            dumw = work.tile([128, 512], FP16)     # HAM warm-up operand

            nc.vector.memset(V[:, :, :, 64:65], 1.0)
            nc.vector.memset(ones16, 1.0)
            nc.vector.memset(dumw, 0.0)

            # ---- HAM warm-up: keep the PE busy while input DMAs fly so the
            # clock gate is already at 2.4 GHz when real matmuls start ----
            for r in range(30):
                psw = psS.tile([128, 2, 512], FP32, tag="score", name="psw")
                nc.tensor.matmul(
                    psw[:, 0, :], dumw[:, 0:128], dumw, start=True, stop=True
                )

            # ---- generator-based TensorE fillers -------------------------
            # filler: dependency-urgent work (QKV chains, norm tails).
            # backlog: flexible work (final projections) drained when the
            # urgent queue is empty, i.e. during ACT-bound stretches.
            filler = deque()
            backlog = deque()

            def pump(n=1):
                for _ in range(n):
                    while filler:
                        try:
                            next(filler[0])
                            break
                        except StopIteration:
                            filler.popleft()
                    else:
                        while backlog:
                            try:
                                next(backlog[0])
                                break
                            except StopIteration:
                                backlog.popleft()
                        else:
                            break

            def drain_fillers():
                while filler or backlog:
                    q = filler if filler else backlog
                    try:
                        next(q[0])
                    except StopIteration:
                        q.popleft()

            def run_now(gen):
                for _ in gen:
                    pass

            def qk_chain(w_t, dst, hp, st):
                ps = psV.tile([128, 512], FP32, tag="proj")
                for c in range(EC):
                    nc.tensor.matmul(
                        ps,
                        w_t[:, c, 128 * hp : 128 * (hp + 1)],
                        xT[:, c, 512 * st : 512 * (st + 1)],
                        start=(c == 0),
                        stop=(c == EC - 1),
                    )
                    if c % 2 == 1 and c < EC - 1:
                        yield
                nc.vector.tensor_copy(
                    out=dst[:, hp, 512 * st : 512 * (st + 1)], in_=ps
                )

            def v_chain(jt):
                ps = psV.tile([128, 512], FP32, tag="proj")
                psd = ps[:, 0:DC]
                for c in range(EC):
                    nc.tensor.matmul(
                        psd,
                        xT[:, c, 128 * jt : 128 * (jt + 1)],
                        wv[:, c, :],
                        start=(c == 0),
                        stop=(c == EC - 1),
                    )
                    if c == 3:
                        yield
                nc.vector.tensor_copy(
                    out=V[:, jt, :, 0:64],
                    in_=psd.rearrange("p (h d) -> p h d", h=HPC),
                )

            def final_proj(st, use_scalar=False, tail=False):
                ob = outs.tile([128, D], FP16, tag="ob")
                for mt in range(2):
                    psf = psV.tile([128, 512], FP32, tag="proj")
                    for hp in range(2):
                        nc.tensor.matmul(
                            psf,
                            outN[:, hp, 128 * st : 128 * (st + 1)],
                            wo[:, hp, 512 * mt : 512 * (mt + 1)],
                            start=(hp == 0),
                            stop=(hp == 1),
                        )
                    if tail and mt == 1:
                        # alternate engines at the tail so the two halves'
                        # psum->sbuf copies run in parallel
                        nc.vector.tensor_copy(
                            out=ob[:, 512 * mt : 512 * (mt + 1)], in_=psf
                        )
                    elif use_scalar or tail:
                        # ScalarE is idle around the small t=0 blocks
                        nc.scalar.copy(
                            out=ob[:, 512 * mt : 512 * (mt + 1)], in_=psf
                        )
                    else:
                        nc.vector.tensor_copy(
                            out=ob[:, 512 * mt : 512 * (mt + 1)], in_=psf
                        )
                    if tail:
                        nc.sync.dma_start(
                            out=out_d[128 * st : 128 * (st + 1),
                                      512 * mt : 512 * (mt + 1)],
                            in_=ob[:, 512 * mt : 512 * (mt + 1)],
                        )
                    yield
                if not tail:
                    nc.sync.dma_start(
                        out=out_d[128 * st : 128 * (st + 1), :], in_=ob
                    )

            def norm_tail(hp, t, psa, Zrh):
                # PE bcast of 1/Z via K=1 matmul against a ones row, then
                # one copy + one mul per head produce normalized out^T
                # straight from the AV psum.  Run inline at jt==1 of the
                # following block so the DVE reciprocal chain has a head
                # start before the bcast matmul enters the in-order PE queue.
                zbs = normp.tile([64, 2, 512], FP16, tag="zbs")
                for hi in range(2):
                    zb = psV.tile([128, 512], FP32, tag="proj")
                    nc.tensor.matmul(
                        zb[0:64, :],
                        ones16[0:1, 0:64],
                        Zrh[0:1, hi, :],
                        start=True,
                        stop=True,
                    )
                    nc.vector.tensor_copy(out=zbs[:, hi, :], in_=zb[0:64, :])
                    nc.vector.tensor_mul(
                        outN[64 * hi : 64 * hi + 64, hp,
                             512 * t : 512 * (t + 1)],
                        psa[hi][0:64, :],
                        zbs[:, hi, :],
                    )
                    yield

            def emit_av(psa, hp, jt, t, njt, UT):
                off = max(0, 128 * jt - 512 * t)
                cn = 512 - off
                for hi in range(2):
                    h = 2 * hp + hi
                    nc.tensor.matmul(
                        psa[hi][:, off : off + cn],
                        V[:, jt, h, :],
                        UT[:, hi, off : off + cn],
                        start=(jt == 0),
                        stop=(jt == njt - 1),
                    )

            def colblock(hp, t, prev_norm=None):
                njt = 4 * t + 4
                psa = [
                    psA.tile([65, 512], FP32, tag="av", name=f"psa{hi}")
                    for hi in range(2)
                ]
                pending = deque()
                for jt in range(njt):
                    # fillers + the ready AV go in the queue BEFORE the
                    # (exp-gated) score matmul so the in-order PE has work
                    # while it waits for ACT.  Exception: the first two jts
                    # go straight to scores so ACT never starves across the
                    # block boundary.
                    if jt >= 2:
                        pump(4 if t == 0 else 3)
                        if len(pending) >= 3:
                            pjt, pUT = pending.popleft()
                            emit_av(psa, hp, pjt, t, njt, pUT)
                    off = max(0, 128 * jt - 512 * t)
                    cn = 512 - off
                    ps = psS.tile([128, 2, 512], FP32, tag="score")
                    for hi in range(2):
                        ho = 64 * hi
                        nc.tensor.matmul(
                            ps[:, hi, 0:cn],
                            KT[ho : ho + 64, hp, 128 * jt : 128 * (jt + 1)],
                            QT[ho : ho + 64, hp, 512 * t + off : 512 * (t + 1)],
                            start=True,
                            stop=True,
                        )
                    UT = utp.tile([128, 2, 512], FP16, tag="ut")
                    nc.scalar.activation(
                        out=UT[:, :, off : off + cn],
                        in_=ps[:, :, 0:cn],
                        func=mybir.ActivationFunctionType.Exp,
                        scale=0.125,
                    )
                    if jt >= 4 * t:
                        # diagonal 128-block: causal {0,1} mask, both heads
                        # (gpsimd, to keep DVE off the exp->AV critical path)
                        nc.gpsimd.tensor_mul(
                            UT[:, :, off : off + 128],
                            UT[:, :, off : off + 128],
                            mask,
                        )
                    if jt == 1 and prev_norm is not None:
                        # emit the previous block's norm muls, THEN unlock
                        # its final projections (emission order = queue
                        # order; fp must never precede its outN writer)
                        pgen, phq, pt = prev_norm
                        run_now(pgen)
                        queue_fp(phq, pt)
                    pending.append((jt, UT))
                while pending:
                    pjt, pUT = pending.popleft()
                    emit_av(psa, hp, pjt, t, njt, pUT)
                # ---- normalization: Z -> partition 0, reciprocal, fp16 ----
                Zs = normp.tile([1, 2, 512], FP32, tag="zs")
                for hi in range(2):
                    nc.vector.tensor_copy(
                        out=Zs[0:1, hi, :], in_=psa[hi][64:65, :]
                    )
                Zr = normp.tile([1, 2, 512], FP32, tag="zr")
                nc.vector.reciprocal_approx_fast(out=Zr, in_=Zs)
                Zrh = normp.tile([1, 2, 512], FP16, tag="zrh")
                nc.vector.tensor_copy(out=Zrh, in_=Zr)
                return norm_tail(hp, t, psa, Zrh)

            # ---- schedule ------------------------------------------------
            done = set()
            tail_fp = []

            def queue_fp(hp, t):
                done.add((hp, t))
                if (1 - hp, t) in done:
                    for st in range(4 * t, 4 * t + 4):
                        if (hp, t) == BLOCKS[-1]:
                            tail_fp.append(st)
                        else:
                            backlog.append(final_proj(st, use_scalar=(t == 3)))

            # head: everything block (0,0) needs, inline
            run_now(qk_chain(wq, QT, 0, 0))
            run_now(qk_chain(wk, KT, 0, 0))
            run_now(v_chain(0))
            run_now(v_chain(1))
            filler.append(v_chain(2))
            filler.append(v_chain(3))

            # fillers each block needs, queued one block ahead
            prefill = {
                (0, 0): [],
                (0, 1): [("qk", 0, 1), ("v", 4), ("v", 5), ("v", 6), ("v", 7)],
                (1, 1): [("qk", 1, 0), ("qk", 1, 1)],
                (0, 2): [("qk", 0, 2), ("v", 8), ("v", 9), ("v", 10), ("v", 11)],
                (1, 2): [("qk", 1, 2)],
                (0, 3): [("qk", 0, 3), ("v", 12), ("v", 13), ("v", 14), ("v", 15)],
                (1, 3): [("qk", 1, 3)],
                (1, 0): [],
            }

            prev_norm = None
            for i, (hp, t) in enumerate(BLOCKS):
                if i + 1 < len(BLOCKS):
                    for item in prefill[BLOCKS[i + 1]]:
                        if item[0] == "qk":
                            _, fhp, fst = item
                            filler.append(qk_chain(wq, QT, fhp, fst))
                            filler.append(qk_chain(wk, KT, fhp, fst))
                        else:
                            filler.append(v_chain(item[1]))
                gen = colblock(hp, t, prev_norm)
                prev_norm = (gen, hp, t)
            gen, hp, t = prev_norm
            run_now(gen)
            queue_fp(hp, t)
            drain_fillers()
            for st in tail_fp:
                run_now(final_proj(st, tail=True))

    nc.compile()
    return nc


_NC = None


def _prep_in_maps(x, W_q, W_k, W_v, W_o):
    x = np.asarray(x, dtype=np.float32)
    W_q = np.asarray(W_q, dtype=np.float32)
    W_k = np.asarray(W_k, dtype=np.float32)
    W_v = np.asarray(W_v, dtype=np.float32)
    W_o = np.asarray(W_o, dtype=np.float32)
    mask01 = np.triu(np.ones((128, 128), dtype=np.float16))
    mask2 = np.ascontiguousarray(np.stack([mask01, mask01], axis=1))
    in_maps = []
    for c in range(NCORES):
        b, g = divmod(c, 4)
        cols = slice(DC * g, DC * (g + 1))
        in_maps.append(
            {
                "xT": np.ascontiguousarray(x[b].T).astype(np.float16),
                "wqT": np.ascontiguousarray(W_q[cols, :].T).astype(np.float16),
                "wkT": np.ascontiguousarray(W_k[cols, :].T).astype(np.float16),
                "wvT": np.ascontiguousarray(W_v[cols, :].T).astype(np.float16),
                "woT": np.ascontiguousarray(W_o[:, cols].T).astype(np.float16),
                "mask": mask2,
            }
        )
    return in_maps


def _run(x, W_q, W_k, W_v, W_o, **spmd_kwargs):
    global _NC
    if _NC is None:
        _NC = _build()
    in_maps = _prep_in_maps(x, W_q, W_k, W_v, W_o)
    res = bass_utils.run_bass_kernel_spmd(
        _NC, in_maps, core_ids=list(range(NCORES)), **spmd_kwargs
    )
    parts = [res.results[c]["out"].astype(np.float32) for c in range(NCORES)]
    out = np.empty((2, S, D), dtype=np.float32)
    for b in range(2):
        out[b] = parts[4 * b] + parts[4 * b + 1] + parts[4 * b + 2] + parts[4 * b + 3]
    return out, res


def kernel(x, W_q, W_k, W_v, W_o):
    out, _ = _run(x, W_q, W_k, W_v, W_o)
    return out
